# revision 5
# baseline (speedup 1.0000x reference)
"""Single-head causal attention (B=4, S=2048, D=1024, dk=128) on 8 TRN2 cores.

Sharding: core c -> batch b=c//2, half h=c%2.
  - h=0 handles query rows [0:512) u [1536:2048), h=1 handles [512:1536)
    (balances causal work: 4+16 vs 8+12 key-tiles per 512-query block).
  - Each core projects the full K/V for its batch (cheaper than an
    intra-pair collective exchange, which measures ~36us on HW).

Precision: qx/kx are fp8e4m3 (halves the score-path HBM bytes; the
projection matmuls run mixed bf16-stationary x fp8-moving), vx and all
weights stay bf16 - fp8 on the V path alone costs ~2.3e-2 max-rel
error, over the 2e-2 budget, while fp8 q+k measures ~1.4e-2.

Layout: the host pre-marshals every tensor into the exact [partition,
block, chunk, col] layout the SBUF tiles use, so each 512-column block
loads as one DMA with a single 4-8 KB contiguous run per partition.
Per-queue DMA throughput is descriptor-rate-limited (~13 ns/descriptor
on HWDGE): 512 B runs cap a queue near 50 GB/s, 4 KB runs near 200+.

Projections contract d_model on the partition dim and emit qT/kT
[dk, s] directly.  Scores are computed transposed ([key, query]) so
the P@V matmul consumes P tiles as the stationary operand and V in
natural [s, dk] layout; a ones-column appended to V makes the same
matmul accumulate the softmax denominators.  Score PSUM tiles span two
banks [128, 1024] (two key tiles) so one ACTIVATE exps both - the
serial ACT chain is the critical path, and halving the instruction
count saves the 352-cycle fixed overhead per ACTIVATE.  The causal
mask is applied as a multiplicative bf16 mask on P, generated on-chip
from a per-core [128, 16] shift table (pairs of key tiles per compare
via an offset iota) so all 8 cores run one identical program.

DMA: loads are split over the three DMA queues (sync/scalar HWDGE +
gpsimd SWDGE) in need-order; the scalar engine issues all its loads up
front so the exp chain is never blocked behind a DMA issue.  Output is
stored per 512-row block in (p q) k layout = 2 KB contiguous per
partition row.
"""

import math

import numpy as np
import ml_dtypes

import concourse.bacc as bacc
import concourse.tile as tile
import concourse.mybir as mybir
from concourse import bass_utils
from concourse.masks import make_identity
from concourse.tile_rust import add_dep_helper

F32 = mybir.dt.float32
BF16 = mybir.dt.bfloat16
FP8 = mybir.dt.float8e4

B, S, DM, DK = 4, 2048, 1024, 128
NCORES = 8
HALF = S // 2  # query rows per core / key columns per pipeline stage
NCH = DM // 128  # d_model chunks
# program-wide causal shape: query block 0 sees key tiles [0, NJ0),
# block 1 sees [0, NJ1); per-core mask data zeroes what's invalid.
NJ0, NJ1 = 8, 16
VW = DK + 1  # v tiles carry a ones-column for the softmax denominator
SCALE = 1.0 / math.sqrt(DK)
WARMUP_MMS = 16
FILLER_MMS = 20

_CACHE = {}


def _build():
    if "nc" in _CACHE:
        return _CACHE["nc"]
    nc = bacc.Bacc("TRN2", target_bir_lowering=False, debug=False, num_devices=NCORES)

    # all activations pre-blocked host-side: [128, blk, chunk, 512]
    qx_in = nc.dram_tensor("qx", [128, 2, NCH, 512], FP8, kind="ExternalInput").ap()
    kx_in = nc.dram_tensor("kx", [128, 4, NCH, 512], FP8, kind="ExternalInput").ap()
    vx_in = nc.dram_tensor("vx", [128, 4, NCH, 512], BF16, kind="ExternalInput").ap()
    wqk_in = nc.dram_tensor("wqk", [128, NCH, 2 * DK], BF16, kind="ExternalInput").ap()
    wv_in = nc.dram_tensor("wv", [128, NCH, DK], BF16, kind="ExternalInput").ap()
    shifts_in = nc.dram_tensor("shifts", [128, 16], F32, kind="ExternalInput").ap()
    out = nc.dram_tensor("out", [HALF, DK], F32, kind="ExternalOutput").ap()

    with tile.TileContext(nc) as tc:
        with tc.tile_pool(name="const", bufs=1) as const:
            ident = const.tile([128, 128], BF16)
            make_identity(nc, ident)

            wqk = const.tile([128, NCH, 2 * DK], BF16, tag="wqk", name="wqk")
            wv = const.tile([128, NCH, DK], BF16, tag="wv", name="wv")
            shifts = const.tile([128, 16], F32)
            qx = const.tile([128, 2, NCH, 512], FP8)
            kx = const.tile([128, 4, NCH, 512], FP8)
            vx = const.tile([128, 4, NCH, 512], BF16)

            # ---- loads: need-ordered across the three DMA queues
            # (sync/scalar HWDGE + gpsimd SWDGE).  One dma_start per
            # 512-col block = 128 descriptors of 4-8 KB contiguous.
            nc.scalar.dma_start(out=wqk, in_=wqk_in)
            nc.scalar.dma_start(out=qx[:, 0], in_=qx_in[:, 0])
            nc.scalar.dma_start(out=qx[:, 1], in_=qx_in[:, 1])
            nc.scalar.dma_start(out=vx[:, 0], in_=vx_in[:, 0])
            nc.scalar.dma_start(out=vx[:, 3, :, 0:256], in_=vx_in[:, 3, :, 0:256])

            nc.sync.dma_start(out=kx[:, 0], in_=kx_in[:, 0])
            nc.sync.dma_start(out=kx[:, 1], in_=kx_in[:, 1])
            nc.sync.dma_start(out=vx[:, 1], in_=vx_in[:, 1])
            nc.sync.dma_start(out=vx[:, 3, :, 256:512], in_=vx_in[:, 3, :, 256:512])

            nc.gpsimd.dma_start(out=shifts, in_=shifts_in)
            nc.gpsimd.dma_start(out=wv, in_=wv_in)
            nc.gpsimd.dma_start(out=kx[:, 2], in_=kx_in[:, 2])
            nc.gpsimd.dma_start(out=kx[:, 3], in_=kx_in[:, 3])
            nc.gpsimd.dma_start(out=vx[:, 2], in_=vx_in[:, 2])

            # ---- causal masks: mask[p, t, c] = (c >= shift[p, t]).
            # shift[t+1] = shift[t] + 128, so one compare against an offset
            # iota produces the (t, t+1) pair in a single [128, 1024] op.
            iota_i = const.tile([128, 1024], mybir.dt.int32)
            nc.gpsimd.iota(iota_i[:, 0:512], pattern=[[1, 512]], base=0,
                           channel_multiplier=0)
            nc.gpsimd.iota(iota_i[:, 512:1024], pattern=[[1, 512]], base=-128,
                           channel_multiplier=0)
            iota2 = const.tile([128, 1024], F32)
            nc.vector.tensor_copy(iota2, iota_i)
            masks_sb = const.tile([128, 16 * 512], BF16)

            def gen_mask_pair(t):
                nc.vector.tensor_scalar(
                    masks_sb[:, t * 512 : (t + 2) * 512],
                    iota2,
                    shifts[:, t : t + 1],
                    None,
                    op0=mybir.AluOpType.is_ge,
                )

            # ---- PE warmup + filler: dummy matmuls keep the HAM clock-gate
            # open while the PE waits for the first loads.
            w_warm = const.tile([128, 512], BF16)
            nc.vector.memset(w_warm, 1.0)
            last_filler = None
            with tc.tile_pool(name="psW", bufs=1, space="PSUM") as psW:
                ps_w = psW.tile([128, 512], F32)
                for _ in range(WARMUP_MMS):
                    nc.tensor.matmul(
                        ps_w[:, 0:128], ident, ident, start=True, stop=True
                    )
                for _ in range(FILLER_MMS):
                    last_filler = nc.tensor.matmul(
                        ps_w, ident, w_warm, start=True, stop=True
                    )

            # ---- persistent projected tensors ----
            qT_sb = const.tile([128, HALF], BF16)
            kTh = [const.tile([128, HALF], BF16, tag=f"kT{h}", name=f"kT{h}") for h in range(2)]
            vTh = [const.tile([128, HALF], BF16, tag=f"vT{h}", name=f"vT{h}") for h in range(2)]
            vsbh = [const.tile([128, NCH, VW], BF16, tag=f"v{h}", name=f"vsb{h}") for h in range(2)]

            with (
                tc.tile_pool(name="psM", bufs=2, space="PSUM") as psM,
                tc.tile_pool(name="psS", bufs=2, space="PSUM") as psS,
                tc.tile_pool(name="psO", bufs=2, space="PSUM") as psO,
                tc.tile_pool(name="pP", bufs=14) as p_pool,
                tc.tile_pool(name="oo", bufs=4) as o_pool,
            ):

                def project_block(wT, k0, xT, dst, dst0, xoff, w=512):
                    """dst[:, dst0:dst0+w] bf16 = W @ X^T[:, xoff:xoff+w]."""
                    blk, off = xoff // 512, xoff % 512
                    acc = psM.tile([128, 512], F32, tag="ps_misc", name="acc")
                    for c in range(NCH):
                        mm = nc.tensor.matmul(
                            acc[:, 0:w],
                            wT[:, c, k0 : k0 + DK],
                            xT[:, blk, c, off : off + w],
                            start=(c == 0),
                            stop=(c == NCH - 1),
                        )
                        if c == 0 and last_filler is not None:
                            add_dep_helper(
                                mm.ins, last_filler.ins, sync=False,
                                reason="run filler first",
                            )
                    nc.vector.tensor_copy(dst[:, dst0 : dst0 + w], acc[:, 0:w])

                def scores_pair(blk, j, masked):
                    """exp(score) for key tiles (j, j+1) x 512 queries of blk.

                    One [128, 1024] PSUM pair, one ACTIVATE, optional mask
                    multiply.  Returns the bf16 p pair tile."""
                    q_cols = slice(blk * 512, (blk + 1) * 512)
                    h = j // NCH
                    ps_s = psS.tile([128, 1024], F32, tag="score")
                    for i in range(2):
                        jl = (j + i) % NCH
                        nc.tensor.matmul(
                            ps_s[:, i * 512 : (i + 1) * 512],
                            kTh[h][:, jl * 128 : (jl + 1) * 128],
                            qT_sb[:, q_cols],
                            start=True,
                            stop=True,
                        )
                    p_t = p_pool.tile([128, 1024], BF16, tag="p")
                    nc.scalar.activation(
                        p_t, ps_s, mybir.ActivationFunctionType.Exp, scale=SCALE
                    )
                    if masked:
                        nc.vector.tensor_mul(
                            p_t, p_t, masks_sb[:, j * 512 : (j + 2) * 512]
                        )
                    return p_t

                def v_natural(h):
                    project_block(wv, 0, vx, vTh[h], 0, xoff=h * HALF)
                    project_block(wv, 0, vx, vTh[h], 512, xoff=h * HALF + 512)
                    for t in range(NCH):
                        ps = psM.tile([128, 128], BF16, tag="ps_misc")
                        nc.tensor.transpose(
                            ps, vTh[h][:, t * 128 : (t + 1) * 128], ident
                        )
                        nc.vector.tensor_copy(vsbh[h][:, t, 0:DK], ps)
                    nc.vector.memset(vsbh[h][:, :, DK : DK + 1], 1.0)

                o_big = [
                    o_pool.tile([128, 4, DK], F32, tag=f"ob{b}", name=f"ob{b}", bufs=1)
                    for b in range(2)
                ]

                def div_out(blk, qs, ps_o):
                    rec = o_pool.tile([128, 1], F32, tag="rec")
                    nc.vector.reciprocal(rec, ps_o[:, DK : DK + 1])
                    nc.vector.tensor_scalar_mul(o_big[blk][:, qs, :], ps_o[:, 0:DK], rec)
                    if qs == 3:
                        r0 = blk * 512
                        ring = nc.scalar if blk == 0 else nc.sync
                        ring.dma_start(
                            out=out[r0 : r0 + 512, :].rearrange(
                                "(p q) k -> p q k", q=4
                            ),
                            in_=o_big[blk],
                        )

                def pv(ps_o, p_pairs, qs, jset, h, start, stop):
                    j0 = jset[0] if isinstance(jset, list) else jset.start
                    for n, j in enumerate(jset):
                        nc.tensor.matmul(
                            ps_o,
                            p_pairs[(j - j0) // 2][
                                :, (j % 2) * 512 + qs * 128 : (j % 2) * 512 + (qs + 1) * 128
                            ],
                            vsbh[h][:, j % NCH, :],
                            start=(start and n == 0),
                            stop=(stop and n == len(jset) - 1),
                        )

                # ---------- pipeline ----------
                # Q projection: block 0 (qx cols 0:512) first so the first
                # score pair only waits on qx blk0 + kx blk0.
                project_block(wqk, 0, qx, qT_sb, 0, xoff=0)
                project_block(wqk, 0, qx, qT_sb, 512, xoff=512)

                # K-projection blocks interleaved with the score pairs (and
                # exps) that consume them, so the serial ACT exp chain starts
                # as soon as the first kT columns exist.
                p0, p1, p1b = [], [], []
                gen_mask_pair(0)
                project_block(wqk, DK, kx, kTh[0], 0, xoff=0, w=256)
                p0.append(scores_pair(0, 0, True))
                p1.append(scores_pair(1, 0, False))
                gen_mask_pair(2)
                project_block(wqk, DK, kx, kTh[0], 256, xoff=256, w=256)
                p0.append(scores_pair(0, 2, True))
                p1.append(scores_pair(1, 2, False))
                gen_mask_pair(4)
                project_block(wqk, DK, kx, kTh[0], 512, xoff=512)
                p0.append(scores_pair(0, 4, True))
                p1.append(scores_pair(1, 4, False))
                gen_mask_pair(6)
                p0.append(scores_pair(0, 6, True))
                p1.append(scores_pair(1, 6, False))

                gen_mask_pair(8)
                gen_mask_pair(10)
                project_block(wqk, DK, kx, kTh[1], 0, xoff=1024)
                p1b.append(scores_pair(1, 8, True))
                p1b.append(scores_pair(1, 10, True))

                v_natural(0)

                ps_o0 = [psO.tile([128, VW], F32, tag="out", name=f"ps_o0_{i}") for i in range(4)]
                for qs in range(4):
                    pv(ps_o0[qs], p0, qs, range(NJ0), 0, True, True)
                    div_out(0, qs, ps_o0[qs])

                gen_mask_pair(12)
                gen_mask_pair(14)
                project_block(wqk, DK, kx, kTh[1], 512, xoff=1536)
                p1b.append(scores_pair(1, 12, True))
                p1b.append(scores_pair(1, 14, True))

                v_natural(1)
                ps_o1 = [psO.tile([128, VW], F32, tag="out", name=f"ps_o1_{i}") for i in range(4)]
                for qs in range(4):
                    pv(ps_o1[qs], p1, qs, range(NCH), 0, True, False)
                for qs in range(4):
                    pv(ps_o1[qs], p1b, qs, range(NCH, NJ1), 1, False, True)
                    div_out(1, qs, ps_o1[qs])

    nc.compile()
    _CACHE["nc"] = nc
    return nc


def _shift_block(h):
    """[128, 16] f32: mask[p, t, c] = (c >= shift) == (key 128t+p <= query qb+c)."""
    qbase = (0, 1536) if h == 0 else (512, 1024)
    p = np.arange(128, dtype=np.float32)[:, None]
    t = np.arange(16, dtype=np.float32)[None, :]
    qb = np.where(t < NJ0, qbase[0], qbase[1])
    return (128.0 * t + p - qb).astype(np.float32)


def _block4(arr, nblk, dtype):
    """[DM, ncols] -> [128, nblk, NCH, 512] matching the SBUF tile layout."""
    return np.ascontiguousarray(
        arr.reshape(NCH, 128, nblk, 512).transpose(1, 2, 0, 3)
    ).astype(dtype)


def kernel(**inputs):
    queries = np.asarray(inputs["queries"], dtype=np.float32)
    keys = np.asarray(inputs["keys"], dtype=np.float32)
    values = np.asarray(inputs["values"], dtype=np.float32)

    nc = _build()
    f8 = ml_dtypes.float8_e4m3fn
    bf = ml_dtypes.bfloat16
    shifts = [_shift_block(0), _shift_block(1)]
    qrows = [np.r_[0:512, 1536:2048], np.r_[512:1536]]
    wT = {
        nm: np.asarray(inputs[nm], dtype=np.float32).T
        for nm in ("Wq", "Wk", "Wv")
    }
    wqk = np.ascontiguousarray(
        np.concatenate([wT["Wq"], wT["Wk"]], axis=1).reshape(NCH, 128, 2 * DK)
        .transpose(1, 0, 2)
    ).astype(bf)
    wv = np.ascontiguousarray(
        wT["Wv"].reshape(NCH, 128, DK).transpose(1, 0, 2)
    ).astype(bf)
    kxs = [_block4(keys[b].T, 4, f8) for b in range(B)]
    vxs = [_block4(values[b].T, 4, bf) for b in range(B)]

    in_maps = []
    for c in range(NCORES):
        b, h = c // 2, c % 2
        in_maps.append(
            {
                "qx": _block4(queries[b][qrows[h]].T, 2, f8),
                "kx": kxs[b],
                "vx": vxs[b],
                "wqk": wqk,
                "wv": wv,
                "shifts": shifts[h],
            }
        )

    res = bass_utils.run_bass_kernel_spmd(
        nc, in_maps, list(range(NCORES)), **_CACHE.get("run_kwargs", {})
    )
    _CACHE["last_result"] = res

    # store layout is (p q): dram row blk*512 + p*4 + qs <- query qs*128 + p
    r = np.arange(512)
    local_q = (r % 4) * 128 + r // 4  # query index within block at dram row r
    perm = np.concatenate([local_q, 512 + local_q])
    out = np.empty((B, S, DK), dtype=np.float32)
    for c in range(NCORES):
        b, h = c // 2, c % 2
        out[b][qrows[h][perm]] = res.results[c]["out"]
    return out


# revision 7
# speedup vs baseline: 1.0334x; 1.0334x over previous
"""Single-head causal attention (B=4, S=2048, D=1024, dk=128) on 8 TRN2 cores.

Sharding: core c -> batch b=c//2, half h=c%2.
  - h=0 handles query rows [0:512) u [1536:2048), h=1 handles [512:1536)
    (balances causal work: 4+16 vs 8+12 key-tiles per 512-query block).
  - Each core projects the full K/V for its batch (cheaper than an
    intra-pair collective exchange, which measures ~36us on HW).

Precision: qx/kx are fp8e4m3 (halves the score-path HBM bytes; the
projection matmuls run mixed bf16-stationary x fp8-moving), vx and all
weights stay bf16 - fp8 on the V path alone costs ~2.3e-2 max-rel
error, over the 2e-2 budget, while fp8 q+k measures ~1.4e-2.

Layout: the host pre-marshals every tensor into the exact [partition,
block, chunk, col] layout the SBUF tiles use, so each 512-column block
loads as one DMA with a single 4-8 KB contiguous run per partition.
Per-queue DMA throughput is descriptor-rate-limited (~13 ns/descriptor
on HWDGE): 512 B runs cap a queue near 50 GB/s, 4 KB runs near 200+.

Projections contract d_model on the partition dim and emit qT/kT
[dk, s] directly.  Scores are computed transposed ([key, query]) so
the P@V matmul consumes P tiles as the stationary operand and V in
natural [s, dk] layout; a ones-column appended to V makes the same
matmul accumulate the softmax denominators.  Score PSUM tiles span two
banks [128, 1024] (two key tiles) so one ACTIVATE exps both - the
serial ACT chain is the critical path, and halving the instruction
count saves the 352-cycle fixed overhead per ACTIVATE.  The causal
mask is applied as a multiplicative bf16 mask on P, generated on-chip
from a per-core [128, 16] shift table (pairs of key tiles per compare
via an offset iota) so all 8 cores run one identical program.

DMA: loads are split over the three DMA queues (sync/scalar HWDGE +
gpsimd SWDGE) in need-order; the scalar engine issues all its loads up
front so the exp chain is never blocked behind a DMA issue.  Output is
stored per 512-row block in (p q) k layout = 2 KB contiguous per
partition row.
"""

import math

import numpy as np
import ml_dtypes

import concourse.bacc as bacc
import concourse.tile as tile
import concourse.mybir as mybir
from concourse import bass_utils
from concourse.masks import make_identity
from concourse.tile_rust import add_dep_helper

F32 = mybir.dt.float32
BF16 = mybir.dt.bfloat16
FP8 = mybir.dt.float8e4

B, S, DM, DK = 4, 2048, 1024, 128
NCORES = 8
HALF = S // 2  # query rows per core / key columns per pipeline stage
NCH = DM // 128  # d_model chunks
# program-wide causal shape: query block 0 sees key tiles [0, NJ0),
# block 1 sees [0, NJ1); per-core mask data zeroes what's invalid.
NJ0, NJ1 = 8, 16
VW = DK + 1  # v tiles carry a ones-column for the softmax denominator
SCALE = 1.0 / math.sqrt(DK)
WARMUP_MMS = 26
FILLER_MMS = 6

_CACHE = {}


def _build():
    if "nc" in _CACHE:
        return _CACHE["nc"]
    nc = bacc.Bacc("TRN2", target_bir_lowering=False, debug=False, num_devices=NCORES)

    # all activations pre-blocked host-side: [128, blk, chunk, 512]
    qx_in = nc.dram_tensor("qx", [128, 2, NCH, 512], FP8, kind="ExternalInput").ap()
    kx_in = nc.dram_tensor("kx", [128, 4, NCH, 512], FP8, kind="ExternalInput").ap()
    vx_in = nc.dram_tensor("vx", [128, 4, NCH, 512], BF16, kind="ExternalInput").ap()
    wqk_in = nc.dram_tensor("wqk", [128, NCH, 2 * DK], BF16, kind="ExternalInput").ap()
    wv_in = nc.dram_tensor("wv", [128, NCH, DK], BF16, kind="ExternalInput").ap()
    shifts_in = nc.dram_tensor("shifts", [128, 16], F32, kind="ExternalInput").ap()
    out = nc.dram_tensor("out", [HALF, DK], F32, kind="ExternalOutput").ap()

    with tile.TileContext(nc) as tc:
        with tc.tile_pool(name="const", bufs=1) as const:
            ident = const.tile([128, 128], BF16)
            make_identity(nc, ident)

            wqk = const.tile([128, NCH, 2 * DK], BF16, tag="wqk", name="wqk")
            wv = const.tile([128, NCH, DK], BF16, tag="wv", name="wv")
            shifts = const.tile([128, 16], F32)
            qx = const.tile([128, 2, NCH, 512], FP8)
            kx = const.tile([128, 4, NCH, 512], FP8)
            vx = const.tile([128, 4, NCH, 512], BF16)

            # ---- loads: need-ordered across the three DMA queues
            # (sync/scalar HWDGE + gpsimd SWDGE).  One dma_start per
            # 512-col block = 128 descriptors of 4-8 KB contiguous.
            nc.scalar.dma_start(out=wqk, in_=wqk_in)
            nc.scalar.dma_start(out=vx[:, 0], in_=vx_in[:, 0])
            nc.scalar.dma_start(out=vx[:, 1], in_=vx_in[:, 1])

            nc.sync.dma_start(out=kx[:, 0], in_=kx_in[:, 0])
            nc.sync.dma_start(out=kx[:, 1], in_=kx_in[:, 1])
            nc.sync.dma_start(out=kx[:, 3], in_=kx_in[:, 3])
            nc.sync.dma_start(out=vx[:, 3], in_=vx_in[:, 3])

            nc.gpsimd.dma_start(out=shifts, in_=shifts_in)
            nc.gpsimd.dma_start(out=qx[:, 0], in_=qx_in[:, 0])
            nc.gpsimd.dma_start(out=qx[:, 1], in_=qx_in[:, 1])
            nc.gpsimd.dma_start(out=wv, in_=wv_in)
            nc.gpsimd.dma_start(out=kx[:, 2], in_=kx_in[:, 2])
            nc.gpsimd.dma_start(out=vx[:, 2], in_=vx_in[:, 2])

            # ---- causal masks: mask[p, t, c] = (c >= shift[p, t]).
            # shift[t+1] = shift[t] + 128, so one compare against an offset
            # iota produces the (t, t+1) pair in a single [128, 1024] op.
            iota_i = const.tile([128, 1024], mybir.dt.int32)
            nc.gpsimd.iota(iota_i[:, 0:512], pattern=[[1, 512]], base=0,
                           channel_multiplier=0)
            nc.gpsimd.iota(iota_i[:, 512:1024], pattern=[[1, 512]], base=-128,
                           channel_multiplier=0)
            iota2 = const.tile([128, 1024], F32)
            nc.vector.tensor_copy(iota2, iota_i)
            masks_sb = const.tile([128, 16 * 512], BF16)

            def gen_mask_pair(t):
                nc.vector.tensor_scalar(
                    masks_sb[:, t * 512 : (t + 2) * 512],
                    iota2,
                    shifts[:, t : t + 1],
                    None,
                    op0=mybir.AluOpType.is_ge,
                )

            # ---- PE warmup + filler: dummy matmuls keep the HAM clock-gate
            # open while the PE waits for the first loads.
            w_warm = const.tile([128, 512], BF16)
            nc.vector.memset(w_warm, 1.0)
            last_filler = None
            with tc.tile_pool(name="psW", bufs=1, space="PSUM") as psW:
                ps_w = psW.tile([128, 512], F32)
                for _ in range(WARMUP_MMS):
                    nc.tensor.matmul(
                        ps_w[:, 0:128], ident, ident, start=True, stop=True
                    )
                for _ in range(FILLER_MMS):
                    last_filler = nc.tensor.matmul(
                        ps_w, ident, w_warm, start=True, stop=True
                    )

            # ---- persistent projected tensors ----
            qT_sb = const.tile([128, HALF], BF16)
            kTh = [const.tile([128, HALF], BF16, tag=f"kT{h}", name=f"kT{h}") for h in range(2)]
            vTh = [const.tile([128, HALF], BF16, tag=f"vT{h}", name=f"vT{h}") for h in range(2)]
            vsbh = [const.tile([128, NCH, VW], BF16, tag=f"v{h}", name=f"vsb{h}") for h in range(2)]

            with (
                tc.tile_pool(name="psM", bufs=2, space="PSUM") as psM,
                tc.tile_pool(name="psS", bufs=2, space="PSUM") as psS,
                tc.tile_pool(name="psO", bufs=2, space="PSUM") as psO,
                tc.tile_pool(name="pP", bufs=14) as p_pool,
                tc.tile_pool(name="oo", bufs=4) as o_pool,
            ):

                def project_block(wT, k0, xT, dst, dst0, xoff, w=512):
                    """dst[:, dst0:dst0+w] bf16 = W @ X^T[:, xoff:xoff+w]."""
                    blk, off = xoff // 512, xoff % 512
                    acc = psM.tile([128, 512], F32, tag="ps_misc", name="acc")
                    for c in range(NCH):
                        mm = nc.tensor.matmul(
                            acc[:, 0:w],
                            wT[:, c, k0 : k0 + DK],
                            xT[:, blk, c, off : off + w],
                            start=(c == 0),
                            stop=(c == NCH - 1),
                        )
                        if c == 0 and last_filler is not None:
                            add_dep_helper(
                                mm.ins, last_filler.ins, sync=False,
                                reason="run filler first",
                            )
                    nc.vector.tensor_copy(dst[:, dst0 : dst0 + w], acc[:, 0:w])

                def scores_pair(blk, j, masked):
                    """exp(score) for key tiles (j, j+1) x 512 queries of blk.

                    One [128, 1024] PSUM pair, one ACTIVATE, optional mask
                    multiply.  Returns the bf16 p pair tile."""
                    q_cols = slice(blk * 512, (blk + 1) * 512)
                    h = j // NCH
                    ps_s = psS.tile([128, 1024], F32, tag="score")
                    for i in range(2):
                        jl = (j + i) % NCH
                        nc.tensor.matmul(
                            ps_s[:, i * 512 : (i + 1) * 512],
                            kTh[h][:, jl * 128 : (jl + 1) * 128],
                            qT_sb[:, q_cols],
                            start=True,
                            stop=True,
                        )
                    p_t = p_pool.tile([128, 1024], BF16, tag="p")
                    nc.scalar.activation(
                        p_t, ps_s, mybir.ActivationFunctionType.Exp, scale=SCALE
                    )
                    if masked:
                        nc.vector.tensor_mul(
                            p_t, p_t, masks_sb[:, j * 512 : (j + 2) * 512]
                        )
                    return p_t

                def v_natural(h):
                    project_block(wv, 0, vx, vTh[h], 0, xoff=h * HALF)
                    project_block(wv, 0, vx, vTh[h], 512, xoff=h * HALF + 512)
                    for t in range(NCH):
                        ps = psM.tile([128, 128], BF16, tag="ps_misc")
                        nc.tensor.transpose(
                            ps, vTh[h][:, t * 128 : (t + 1) * 128], ident
                        )
                        nc.vector.tensor_copy(vsbh[h][:, t, 0:DK], ps)
                    nc.vector.memset(vsbh[h][:, :, DK : DK + 1], 1.0)

                o_big = [
                    o_pool.tile([128, 4, DK], F32, tag=f"ob{b}", name=f"ob{b}", bufs=1)
                    for b in range(2)
                ]

                out4 = out.rearrange("(b p q) k -> b p q k", q=4, p=128)

                def div_out(blk, qs, ps_o):
                    rec = o_pool.tile([128, 1], F32, tag="rec")
                    nc.vector.reciprocal(rec, ps_o[:, DK : DK + 1])
                    nc.vector.tensor_scalar_mul(o_big[blk][:, qs, :], ps_o[:, 0:DK], rec)
                    if qs == 3 and blk == 0:
                        # issued on sync: emitted here = before sync's later
                        # vx loads retire, but sync has nothing else to do
                        nc.sync.dma_start(out=out4[0], in_=o_big[0])

                def pv(ps_o, p_pairs, qs, jset, h, start, stop):
                    j0 = jset[0] if isinstance(jset, list) else jset.start
                    for n, j in enumerate(jset):
                        nc.tensor.matmul(
                            ps_o,
                            p_pairs[(j - j0) // 2][
                                :, (j % 2) * 512 + qs * 128 : (j % 2) * 512 + (qs + 1) * 128
                            ],
                            vsbh[h][:, j % NCH, :],
                            start=(start and n == 0),
                            stop=(stop and n == len(jset) - 1),
                        )

                # ---------- pipeline ----------
                # Q projection: block 0 (qx cols 0:512) first so the first
                # score pair only waits on qx blk0 + kx blk0.
                with nc.named_scope("q_proj"):
                    project_block(wqk, 0, qx, qT_sb, 0, xoff=0)
                    project_block(wqk, 0, qx, qT_sb, 512, xoff=512)

                # K-projection blocks interleaved with the score pairs (and
                # exps) that consume them, so the serial ACT exp chain starts
                # as soon as the first kT columns exist.
                p0, p1, p1b = [], [], []
                with nc.named_scope("sc_a"):
                    gen_mask_pair(0)
                    project_block(wqk, DK, kx, kTh[0], 0, xoff=0, w=256)
                    p0.append(scores_pair(0, 0, True))
                    p1.append(scores_pair(1, 0, False))
                    gen_mask_pair(2)
                    project_block(wqk, DK, kx, kTh[0], 256, xoff=256, w=256)
                    p0.append(scores_pair(0, 2, True))
                    p1.append(scores_pair(1, 2, False))
                with nc.named_scope("sc_b"):
                    gen_mask_pair(4)
                    project_block(wqk, DK, kx, kTh[0], 512, xoff=512)
                    p0.append(scores_pair(0, 4, True))
                    p1.append(scores_pair(1, 4, False))
                    gen_mask_pair(6)
                    p0.append(scores_pair(0, 6, True))
                    p1.append(scores_pair(1, 6, False))

                with nc.named_scope("sc_c"):
                    gen_mask_pair(8)
                    gen_mask_pair(10)
                    project_block(wqk, DK, kx, kTh[1], 0, xoff=1024)
                    p1b.append(scores_pair(1, 8, True))
                    p1b.append(scores_pair(1, 10, True))

                with nc.named_scope("vnat0"):
                    v_natural(0)

                ps_o0 = [psO.tile([128, VW], F32, tag="out", name=f"ps_o0_{i}") for i in range(4)]
                with nc.named_scope("pv0"):
                    for qs in range(4):
                        pv(ps_o0[qs], p0, qs, range(NJ0), 0, True, True)
                        div_out(0, qs, ps_o0[qs])

                with nc.named_scope("sc_d"):
                    gen_mask_pair(12)
                    gen_mask_pair(14)
                    project_block(wqk, DK, kx, kTh[1], 512, xoff=1536)
                    p1b.append(scores_pair(1, 12, True))
                    p1b.append(scores_pair(1, 14, True))

                with nc.named_scope("vnat1"):
                    v_natural(1)
                ps_o1 = [psO.tile([128, VW], F32, tag="out", name=f"ps_o1_{i}") for i in range(4)]
                with nc.named_scope("pv1"):
                    for qs in range(4):
                        pv(ps_o1[qs], p1, qs, range(NCH), 0, True, False)
                with nc.named_scope("pv1b"):
                    for qs in range(4):
                        pv(ps_o1[qs], p1b, qs, range(NCH, NJ1), 1, False, True)
                        div_out(1, qs, ps_o1[qs])
                with nc.named_scope("store1"):
                    nc.scalar.dma_start(out=out4[1, :, 0:2], in_=o_big[1][:, 0:2])
                    nc.sync.dma_start(out=out4[1, :, 2:4], in_=o_big[1][:, 2:4])

    nc.compile()
    _CACHE["nc"] = nc
    return nc


def _shift_block(h):
    """[128, 16] f32: mask[p, t, c] = (c >= shift) == (key 128t+p <= query qb+c)."""
    qbase = (0, 1536) if h == 0 else (512, 1024)
    p = np.arange(128, dtype=np.float32)[:, None]
    t = np.arange(16, dtype=np.float32)[None, :]
    qb = np.where(t < NJ0, qbase[0], qbase[1])
    return (128.0 * t + p - qb).astype(np.float32)


def _block4(arr, nblk, dtype):
    """[DM, ncols] -> [128, nblk, NCH, 512] matching the SBUF tile layout."""
    return np.ascontiguousarray(
        arr.reshape(NCH, 128, nblk, 512).transpose(1, 2, 0, 3)
    ).astype(dtype)


def kernel(**inputs):
    queries = np.asarray(inputs["queries"], dtype=np.float32)
    keys = np.asarray(inputs["keys"], dtype=np.float32)
    values = np.asarray(inputs["values"], dtype=np.float32)

    nc = _build()
    f8 = ml_dtypes.float8_e4m3fn
    bf = ml_dtypes.bfloat16
    shifts = [_shift_block(0), _shift_block(1)]
    qrows = [np.r_[0:512, 1536:2048], np.r_[512:1536]]
    wT = {
        nm: np.asarray(inputs[nm], dtype=np.float32).T
        for nm in ("Wq", "Wk", "Wv")
    }
    wqk = np.ascontiguousarray(
        np.concatenate([wT["Wq"], wT["Wk"]], axis=1).reshape(NCH, 128, 2 * DK)
        .transpose(1, 0, 2)
    ).astype(bf)
    wv = np.ascontiguousarray(
        wT["Wv"].reshape(NCH, 128, DK).transpose(1, 0, 2)
    ).astype(bf)
    kxs = [_block4(keys[b].T, 4, f8) for b in range(B)]
    vxs = [_block4(values[b].T, 4, bf) for b in range(B)]

    in_maps = []
    for c in range(NCORES):
        b, h = c // 2, c % 2
        in_maps.append(
            {
                "qx": _block4(queries[b][qrows[h]].T, 2, f8),
                "kx": kxs[b],
                "vx": vxs[b],
                "wqk": wqk,
                "wv": wv,
                "shifts": shifts[h],
            }
        )

    res = bass_utils.run_bass_kernel_spmd(
        nc, in_maps, list(range(NCORES)), **_CACHE.get("run_kwargs", {})
    )
    _CACHE["last_result"] = res

    # store layout is (p q): dram row blk*512 + p*4 + qs <- query qs*128 + p
    r = np.arange(512)
    local_q = (r % 4) * 128 + r // 4  # query index within block at dram row r
    perm = np.concatenate([local_q, 512 + local_q])
    out = np.empty((B, S, DK), dtype=np.float32)
    for c in range(NCORES):
        b, h = c // 2, c % 2
        out[b][qrows[h][perm]] = res.results[c]["out"]
    return out


# revision 8
# speedup vs baseline: 1.0741x; 1.0394x over previous
"""Single-head causal attention (B=4, S=2048, D=1024, dk=128) on 8 TRN2 cores.

Sharding: core c -> batch b=c//2, half h=c%2.
  - h=0 handles query rows [0:512) u [1536:2048), h=1 handles [512:1536)
    (balances causal work: 4+16 vs 8+12 key-tiles per 512-query block).
  - Each core projects the full K/V for its batch (cheaper than an
    intra-pair collective exchange, which measures ~36us on HW).

Precision: qx/kx are fp8e4m3 (halves the score-path HBM bytes; the
projection matmuls run mixed bf16-stationary x fp8-moving), vx and all
weights stay bf16 - fp8 on the V path alone costs ~2.3e-2 max-rel
error, over the 2e-2 budget, while fp8 q+k measures ~1.4e-2.

Layout: the host pre-marshals every tensor into the exact [partition,
block, chunk, col] layout the SBUF tiles use, so each 512-column block
loads as one DMA with a single 4-8 KB contiguous run per partition.
Per-queue DMA throughput is descriptor-rate-limited (~13 ns/descriptor
on HWDGE): 512 B runs cap a queue near 50 GB/s, 4 KB runs near 200+.

Projections contract d_model on the partition dim and emit qT/kT
[dk, s] directly.  Scores are computed transposed ([key, query]) so
the P@V matmul consumes P tiles as the stationary operand and V in
natural [s, dk] layout; a ones-column appended to V makes the same
matmul accumulate the softmax denominators.  Score PSUM tiles span two
banks [128, 1024] (two key tiles) so one ACTIVATE exps both - the
serial ACT chain is the critical path, and halving the instruction
count saves the 352-cycle fixed overhead per ACTIVATE.  The causal
mask is applied as a multiplicative bf16 mask on P, generated on-chip
from a per-core [128, 16] shift table (pairs of key tiles per compare
via an offset iota) so all 8 cores run one identical program.

DMA: loads are split over the three DMA queues (sync/scalar HWDGE +
gpsimd SWDGE) in need-order; the scalar engine issues all its loads up
front so the exp chain is never blocked behind a DMA issue.  Output is
stored per 512-row block in (p q) k layout = 2 KB contiguous per
partition row.
"""

import math

import numpy as np
import ml_dtypes

import concourse.bacc as bacc
import concourse.tile as tile
import concourse.mybir as mybir
from concourse import bass_utils
from concourse.masks import make_identity
from concourse.tile_rust import add_dep_helper

F32 = mybir.dt.float32
BF16 = mybir.dt.bfloat16
FP8 = mybir.dt.float8e4

B, S, DM, DK = 4, 2048, 1024, 128
NCORES = 8
HALF = S // 2  # query rows per core / key columns per pipeline stage
NCH = DM // 128  # d_model chunks
# program-wide causal shape: query block 0 sees key tiles [0, NJ0),
# block 1 sees [0, NJ1); per-core mask data zeroes what's invalid.
NJ0, NJ1 = 8, 16
VW = DK + 1  # v tiles carry a ones-column for the softmax denominator
WSC = 16.0  # wq/wk pre-scaled into fp8's normal range
SCALE = 1.0 / (math.sqrt(DK) * WSC * WSC)
WARMUP_MMS = 26
FILLER_MMS = 6

_CACHE = {}


def _build():
    if "nc" in _CACHE:
        return _CACHE["nc"]
    nc = bacc.Bacc("TRN2", target_bir_lowering=False, debug=False, num_devices=NCORES)

    # all activations pre-blocked host-side: [128, blk, chunk, 512]
    qx_in = nc.dram_tensor("qx", [128, 2, NCH, 512], FP8, kind="ExternalInput").ap()
    kx_in = nc.dram_tensor("kx", [128, 4, NCH, 512], FP8, kind="ExternalInput").ap()
    vx_in = nc.dram_tensor("vx", [128, 4, NCH, 512], BF16, kind="ExternalInput").ap()
    wqk_in = nc.dram_tensor("wqk", [128, NCH, 2 * DK], FP8, kind="ExternalInput").ap()
    wv_in = nc.dram_tensor("wv", [128, NCH, DK], BF16, kind="ExternalInput").ap()
    shifts_in = nc.dram_tensor("shifts", [128, 16], F32, kind="ExternalInput").ap()
    out = nc.dram_tensor("out", [HALF, DK], F32, kind="ExternalOutput").ap()

    with tile.TileContext(nc) as tc:
        with tc.tile_pool(name="const", bufs=1) as const:
            ident = const.tile([128, 128], BF16)
            make_identity(nc, ident)

            wqk = const.tile([128, NCH, 2 * DK], FP8, tag="wqk", name="wqk")
            wv = const.tile([128, NCH, DK], BF16, tag="wv", name="wv")
            shifts = const.tile([128, 16], F32)
            qx = const.tile([128, 2, NCH, 512], FP8)
            kx = const.tile([128, 4, NCH, 512], FP8)
            vx = const.tile([128, 4, NCH, 512], BF16)

            # ---- loads: need-ordered across the three DMA queues
            # (sync/scalar HWDGE + gpsimd SWDGE).  One dma_start per
            # 512-col block = 128 descriptors of 4-8 KB contiguous.
            # iotas first: anything ahead of gpsimd's dma_starts delays
            # SWDGE descriptor generation (runs on the GpSimd engine).
            iota_i = const.tile([128, 1024], mybir.dt.int32)
            nc.gpsimd.iota(iota_i[:, 0:512], pattern=[[1, 512]], base=0,
                           channel_multiplier=0)
            nc.gpsimd.iota(iota_i[:, 512:1024], pattern=[[1, 512]], base=-128,
                           channel_multiplier=0)

            nc.scalar.dma_start(out=wqk, in_=wqk_in)
            nc.scalar.dma_start(out=qx[:, 0], in_=qx_in[:, 0])
            nc.scalar.dma_start(out=qx[:, 1], in_=qx_in[:, 1])
            nc.scalar.dma_start(out=kx[:, 2], in_=kx_in[:, 2])
            nc.scalar.dma_start(out=vx[:, 0], in_=vx_in[:, 0])
            nc.scalar.dma_start(out=vx[:, 3, :, 0:256], in_=vx_in[:, 3, :, 0:256])

            nc.sync.dma_start(out=kx[:, 0], in_=kx_in[:, 0])
            nc.sync.dma_start(out=kx[:, 1], in_=kx_in[:, 1])
            nc.sync.dma_start(out=kx[:, 3], in_=kx_in[:, 3])
            nc.sync.dma_start(out=vx[:, 1], in_=vx_in[:, 1])

            nc.gpsimd.dma_start(out=shifts, in_=shifts_in)
            nc.gpsimd.dma_start(out=wv, in_=wv_in)
            nc.gpsimd.dma_start(out=vx[:, 2], in_=vx_in[:, 2])
            nc.gpsimd.dma_start(out=vx[:, 3, :, 256:512], in_=vx_in[:, 3, :, 256:512])

            # ---- causal masks: mask[p, t, c] = (c >= shift[p, t]).
            # shift[t+1] = shift[t] + 128, so one compare against an offset
            # iota produces the (t, t+1) pair in a single [128, 1024] op.
            iota2 = const.tile([128, 1024], F32)
            nc.vector.tensor_copy(iota2, iota_i)
            masks_sb = const.tile([128, 16 * 512], BF16)

            def gen_mask_pair(t):
                nc.vector.tensor_scalar(
                    masks_sb[:, t * 512 : (t + 2) * 512],
                    iota2,
                    shifts[:, t : t + 1],
                    None,
                    op0=mybir.AluOpType.is_ge,
                )

            # ---- PE warmup + filler: dummy matmuls keep the HAM clock-gate
            # open while the PE waits for the first loads.
            w_warm = const.tile([128, 512], BF16)
            nc.vector.memset(w_warm, 1.0)
            last_filler = None
            with tc.tile_pool(name="psW", bufs=1, space="PSUM") as psW:
                ps_w = psW.tile([128, 512], F32)
                for _ in range(WARMUP_MMS):
                    nc.tensor.matmul(
                        ps_w[:, 0:128], ident, ident, start=True, stop=True
                    )
                for _ in range(FILLER_MMS):
                    last_filler = nc.tensor.matmul(
                        ps_w, ident, w_warm, start=True, stop=True
                    )

            # ---- persistent projected tensors ----
            qT_sb = const.tile([128, HALF], BF16)
            kTh = [const.tile([128, HALF], BF16, tag=f"kT{h}", name=f"kT{h}") for h in range(2)]
            vTh = [const.tile([128, HALF], BF16, tag=f"vT{h}", name=f"vT{h}") for h in range(2)]
            vsbh = [const.tile([128, NCH, VW], BF16, tag=f"v{h}", name=f"vsb{h}") for h in range(2)]

            with (
                tc.tile_pool(name="psM", bufs=2, space="PSUM") as psM,
                tc.tile_pool(name="psS", bufs=2, space="PSUM") as psS,
                tc.tile_pool(name="psO", bufs=2, space="PSUM") as psO,
                tc.tile_pool(name="pP", bufs=14) as p_pool,
                tc.tile_pool(name="oo", bufs=4) as o_pool,
            ):

                def project_block(wT, k0, xT, dst, dst0, xoff, w=512):
                    """dst[:, dst0:dst0+w] bf16 = W @ X^T[:, xoff:xoff+w]."""
                    blk, off = xoff // 512, xoff % 512
                    acc = psM.tile([128, 512], F32, tag="ps_misc", name="acc")
                    for c in range(NCH):
                        mm = nc.tensor.matmul(
                            acc[:, 0:w],
                            wT[:, c, k0 : k0 + DK],
                            xT[:, blk, c, off : off + w],
                            start=(c == 0),
                            stop=(c == NCH - 1),
                        )
                        if c == 0 and last_filler is not None:
                            add_dep_helper(
                                mm.ins, last_filler.ins, sync=False,
                                reason="run filler first",
                            )
                    nc.vector.tensor_copy(dst[:, dst0 : dst0 + w], acc[:, 0:w])

                def scores_pair(blk, j, masked):
                    """exp(score) for key tiles (j, j+1) x 512 queries of blk.

                    One [128, 1024] PSUM pair, one ACTIVATE, optional mask
                    multiply.  Returns the bf16 p pair tile."""
                    q_cols = slice(blk * 512, (blk + 1) * 512)
                    h = j // NCH
                    ps_s = psS.tile([128, 1024], F32, tag="score")
                    for i in range(2):
                        jl = (j + i) % NCH
                        nc.tensor.matmul(
                            ps_s[:, i * 512 : (i + 1) * 512],
                            kTh[h][:, jl * 128 : (jl + 1) * 128],
                            qT_sb[:, q_cols],
                            start=True,
                            stop=True,
                        )
                    p_t = p_pool.tile([128, 1024], BF16, tag="p")
                    nc.scalar.activation(
                        p_t, ps_s, mybir.ActivationFunctionType.Exp, scale=SCALE
                    )
                    if masked:
                        nc.vector.tensor_mul(
                            p_t, p_t, masks_sb[:, j * 512 : (j + 2) * 512]
                        )
                    return p_t

                def v_natural(h):
                    project_block(wv, 0, vx, vTh[h], 0, xoff=h * HALF)
                    project_block(wv, 0, vx, vTh[h], 512, xoff=h * HALF + 512)
                    for t in range(NCH):
                        ps = psM.tile([128, 128], BF16, tag="ps_misc")
                        nc.tensor.transpose(
                            ps, vTh[h][:, t * 128 : (t + 1) * 128], ident
                        )
                        nc.vector.tensor_copy(vsbh[h][:, t, 0:DK], ps)
                    nc.vector.memset(vsbh[h][:, :, DK : DK + 1], 1.0)

                o_big = [
                    o_pool.tile([128, 4, DK], F32, tag=f"ob{b}", name=f"ob{b}", bufs=1)
                    for b in range(2)
                ]

                out4 = out.rearrange("(b p q) k -> b p q k", q=4, p=128)

                def div_out(blk, qs, ps_o):
                    rec = o_pool.tile([128, 1], F32, tag="rec")
                    nc.vector.reciprocal(rec, ps_o[:, DK : DK + 1])
                    nc.vector.tensor_scalar_mul(o_big[blk][:, qs, :], ps_o[:, 0:DK], rec)
                    if qs == 3 and blk == 0:
                        # issued on sync: emitted here = before sync's later
                        # vx loads retire, but sync has nothing else to do
                        nc.sync.dma_start(out=out4[0], in_=o_big[0])

                def pv(ps_o, p_pairs, qs, jset, h, start, stop):
                    j0 = jset[0] if isinstance(jset, list) else jset.start
                    for n, j in enumerate(jset):
                        nc.tensor.matmul(
                            ps_o,
                            p_pairs[(j - j0) // 2][
                                :, (j % 2) * 512 + qs * 128 : (j % 2) * 512 + (qs + 1) * 128
                            ],
                            vsbh[h][:, j % NCH, :],
                            start=(start and n == 0),
                            stop=(stop and n == len(jset) - 1),
                        )

                # ---------- pipeline ----------
                # Q projection: block 0 (qx cols 0:512) first so the first
                # score pair only waits on qx blk0 + kx blk0.
                with nc.named_scope("q_proj"):
                    project_block(wqk, 0, qx, qT_sb, 0, xoff=0)
                    project_block(wqk, 0, qx, qT_sb, 512, xoff=512)

                # K-projection blocks interleaved with the score pairs (and
                # exps) that consume them, so the serial ACT exp chain starts
                # as soon as the first kT columns exist.
                p0, p1, p1b = [], [], []
                with nc.named_scope("sc_a"):
                    gen_mask_pair(0)
                    project_block(wqk, DK, kx, kTh[0], 0, xoff=0, w=256)
                    p0.append(scores_pair(0, 0, True))
                    p1.append(scores_pair(1, 0, False))
                    gen_mask_pair(2)
                    project_block(wqk, DK, kx, kTh[0], 256, xoff=256, w=256)
                    p0.append(scores_pair(0, 2, True))
                    p1.append(scores_pair(1, 2, False))
                with nc.named_scope("sc_b"):
                    gen_mask_pair(4)
                    project_block(wqk, DK, kx, kTh[0], 512, xoff=512)
                    p0.append(scores_pair(0, 4, True))
                    p1.append(scores_pair(1, 4, False))
                    gen_mask_pair(6)
                    p0.append(scores_pair(0, 6, True))
                    p1.append(scores_pair(1, 6, False))

                with nc.named_scope("sc_c"):
                    gen_mask_pair(8)
                    gen_mask_pair(10)
                    project_block(wqk, DK, kx, kTh[1], 0, xoff=1024)
                    p1b.append(scores_pair(1, 8, True))
                    p1b.append(scores_pair(1, 10, True))

                with nc.named_scope("vnat0"):
                    v_natural(0)

                ps_o0 = [psO.tile([128, VW], F32, tag="out", name=f"ps_o0_{i}") for i in range(4)]
                with nc.named_scope("pv0"):
                    for qs in range(4):
                        pv(ps_o0[qs], p0, qs, range(NJ0), 0, True, True)
                        div_out(0, qs, ps_o0[qs])

                with nc.named_scope("sc_d"):
                    gen_mask_pair(12)
                    gen_mask_pair(14)
                    project_block(wqk, DK, kx, kTh[1], 512, xoff=1536)
                    p1b.append(scores_pair(1, 12, True))
                    p1b.append(scores_pair(1, 14, True))

                with nc.named_scope("vnat1"):
                    v_natural(1)
                ps_o1 = [psO.tile([128, VW], F32, tag="out", name=f"ps_o1_{i}") for i in range(4)]
                with nc.named_scope("pv1"):
                    for qs in range(4):
                        pv(ps_o1[qs], p1, qs, range(NCH), 0, True, False)
                with nc.named_scope("pv1b"):
                    for qs in range(4):
                        pv(ps_o1[qs], p1b, qs, range(NCH, NJ1), 1, False, True)
                        div_out(1, qs, ps_o1[qs])
                with nc.named_scope("store1"):
                    nc.scalar.dma_start(out=out4[1, :, 0:2], in_=o_big[1][:, 0:2])
                    nc.sync.dma_start(out=out4[1, :, 2:4], in_=o_big[1][:, 2:4])

    nc.compile()
    _CACHE["nc"] = nc
    return nc


def _shift_block(h):
    """[128, 16] f32: mask[p, t, c] = (c >= shift) == (key 128t+p <= query qb+c)."""
    qbase = (0, 1536) if h == 0 else (512, 1024)
    p = np.arange(128, dtype=np.float32)[:, None]
    t = np.arange(16, dtype=np.float32)[None, :]
    qb = np.where(t < NJ0, qbase[0], qbase[1])
    return (128.0 * t + p - qb).astype(np.float32)


def _block4(arr, nblk, dtype):
    """[DM, ncols] -> [128, nblk, NCH, 512] matching the SBUF tile layout."""
    return np.ascontiguousarray(
        arr.reshape(NCH, 128, nblk, 512).transpose(1, 2, 0, 3)
    ).astype(dtype)


def kernel(**inputs):
    queries = np.asarray(inputs["queries"], dtype=np.float32)
    keys = np.asarray(inputs["keys"], dtype=np.float32)
    values = np.asarray(inputs["values"], dtype=np.float32)

    nc = _build()
    f8 = ml_dtypes.float8_e4m3fn
    bf = ml_dtypes.bfloat16
    shifts = [_shift_block(0), _shift_block(1)]
    qrows = [np.r_[0:512, 1536:2048], np.r_[512:1536]]
    wT = {
        nm: np.asarray(inputs[nm], dtype=np.float32).T
        for nm in ("Wq", "Wk", "Wv")
    }
    wqk = np.ascontiguousarray(
        np.concatenate([wT["Wq"], wT["Wk"]], axis=1).reshape(NCH, 128, 2 * DK)
        .transpose(1, 0, 2) * WSC
    ).astype(f8)
    wv = np.ascontiguousarray(
        wT["Wv"].reshape(NCH, 128, DK).transpose(1, 0, 2)
    ).astype(bf)
    kxs = [_block4(keys[b].T, 4, f8) for b in range(B)]
    vxs = [_block4(values[b].T, 4, bf) for b in range(B)]

    in_maps = []
    for c in range(NCORES):
        b, h = c // 2, c % 2
        in_maps.append(
            {
                "qx": _block4(queries[b][qrows[h]].T, 2, f8),
                "kx": kxs[b],
                "vx": vxs[b],
                "wqk": wqk,
                "wv": wv,
                "shifts": shifts[h],
            }
        )

    res = bass_utils.run_bass_kernel_spmd(
        nc, in_maps, list(range(NCORES)), **_CACHE.get("run_kwargs", {})
    )
    _CACHE["last_result"] = res

    # store layout is (p q): dram row blk*512 + p*4 + qs <- query qs*128 + p
    r = np.arange(512)
    local_q = (r % 4) * 128 + r // 4  # query index within block at dram row r
    perm = np.concatenate([local_q, 512 + local_q])
    out = np.empty((B, S, DK), dtype=np.float32)
    for c in range(NCORES):
        b, h = c // 2, c % 2
        out[b][qrows[h][perm]] = res.results[c]["out"]
    return out
